# revision 2
# baseline (speedup 1.0000x reference)
"""Trainium2 Bass kernel for nn_HAO_42923903156378 (gnn_message_passing).

Strategy (8 NeuronCores, tensor-parallel over the two huge matvecs):
  - Small graph-conv ops (compress layer, t/s hyper-GCN, adjacency sigmoid)
    are replicated on every core.
  - Wout (D,D = 22016^2, 1.94 GB) is sharded row-wise: core k computes
    out[2752k : 2752(k+1)] = relu(Wout[rows_k] @ vec).  The weights are
    shipped pre-transposed and row-permuted so the on-device activation
    layout matches exactly: the matvec streams fp32 weight tiles [128, 512]
    through the PE with the activation chunk [128,1] stationary.
  - Wfcn (8192 x 22016, 0.72 GB) is sharded along the *contraction* dim:
    core k computes partial_k = Wfcn[:, rows_k] @ out[rows_k]; the host sums
    the 8 partials (+bias, relu).  No device collectives needed.
  - t_out/s_out sigmoids are computed on device; curvs passes through.

Inputs are full/unsharded; all sharding happens inside kernel().
"""

import numpy as np

import concourse.bass as bass
import concourse.mybir as mybir
from concourse import bacc
from concourse.tile import TileContext
from concourse import bass_utils

F = 128
N = 64
CD = 8
D = CD * F + CD * N + F * F + N * N  # 22016
NCORES = 8
RPC = D // NCORES            # 2752 rows of Wout per core
C_CHUNKS = D // 128          # 172 contraction chunks
OUT_CHUNKS = 6               # 5x512 + 192 = 2752
PH2_K = 2816                 # 2752 padded to 22*128
PH2_KC = PH2_K // 128        # 22
NF = N * F                   # 8192

_f32 = mybir.dt.float32

# smalls layout (columns)
_S_XT = 0        # x.T               [128, 64]
_S_WCT = 64      # Wc.T              [128, 128]
_S_WTT = 192     # Wt.T              [128, 8]
_S_WST = 200     # Ws.T (pad)        [128, 8]
_S_WTADJT = 208  # Wtadj.T (pad)     [128, 64]
_S_TADJT = 272   # t_adj_hyp.T (pad) [128, 64]
_S_WSADJT = 336  # Wsadj.T           [128, 128]
_S_SADJT = 464   # s_adj_hyp.T       [128, 128]
_S_IDENT = 592   # identity          [128, 128]
_S_COLS = 720

_nc_cache = {}


def _build_nc():
    if "nc" in _nc_cache:
        return _nc_cache["nc"]

    nc = bacc.Bacc("TRN2")
    smalls = nc.dram_tensor("smalls", [128, _S_COLS], _f32, kind="ExternalInput")
    wbig = nc.dram_tensor("wbig", [D, RPC], _f32, kind="ExternalInput")
    wf = nc.dram_tensor("wf", [PH2_K, NF], _f32, kind="ExternalInput")
    out_sig = nc.dram_tensor("out_sig", [128, PH2_KC], _f32, kind="ExternalOutput")
    xpart = nc.dram_tensor("xpart", [1, NF], _f32, kind="ExternalOutput")

    with TileContext(nc) as tc:
        with (
            tc.tile_pool(name="sb_fix", bufs=1) as sbf,
            tc.tile_pool(name="strip1", bufs=6) as sp1,
            tc.tile_pool(name="strip2", bufs=4) as sp2,
            tc.tile_pool(name="psum", bufs=1, space="PSUM") as pp,
        ):
            PT = ["pa", "pb", "pc", "pd", "pe", "pf", "pg", "ph"]  # 8 banks

            smalls_sb = sbf.tile([128, _S_COLS], _f32)
            nc.sync.dma_start(out=smalls_sb[:], in_=smalls[:])

            def sm(lo, n):
                return smalls_sb[:, lo:lo + n]

            # ---------- small graph ops (replicated) ----------
            with nc.named_scope("small"):
                xc_pad = sbf.tile([128, 128], _f32)    # xc, zero-padded rows 64:
                xcT = sbf.tile([128, 64], _f32)        # xc.T
                adjtT_pad = sbf.tile([128, 64], _f32)  # adj_t.T, zero-padded
                adjsT = sbf.tile([128, 128], _f32)     # adj_s.T
                w_pad = sbf.tile([128, 8], _f32)       # xc@Wt.T, zero-padded
                u_sb = sbf.tile([128, 8], _f32)        # xc.T@Ws.T
                tf_sb = sbf.tile([64, 8], _f32)
                sf_sb = sbf.tile([128, 8], _f32)
                vec_tile = sbf.tile([128, C_CHUNKS], _f32)

                nc.vector.memset(xc_pad[:], 0.0)
                nc.vector.memset(adjtT_pad[:], 0.0)
                nc.vector.memset(w_pad[:], 0.0)

                Relu = mybir.ActivationFunctionType.Relu
                Sig = mybir.ActivationFunctionType.Sigmoid
                Copy = mybir.ActivationFunctionType.Copy

                xc_ps = pp.tile([64, 128], _f32, tag=PT[0], name="xc_ps")
                nc.tensor.matmul(xc_ps[:], sm(_S_XT, 64), sm(_S_WCT, 128))
                xcT_ps = pp.tile([128, 64], _f32, tag=PT[1], name="xcT_ps")
                nc.tensor.matmul(xcT_ps[:], sm(_S_WCT, 128), sm(_S_XT, 64))
                adjtT_ps = pp.tile([64, 64], _f32, tag=PT[2], name="adjtT_ps")
                nc.tensor.matmul(adjtT_ps[:], sm(_S_WTADJT, 64), sm(_S_TADJT, 64))
                adjsT_ps = pp.tile([128, 128], _f32, tag=PT[3], name="adjsT_ps")
                nc.tensor.matmul(adjsT_ps[:], sm(_S_WSADJT, 128), sm(_S_SADJT, 128))

                nc.scalar.activation(xc_pad[0:64, :], xc_ps[:], Relu)
                nc.scalar.activation(xcT[:], xcT_ps[:], Relu)
                nc.scalar.activation(adjtT_pad[0:64, :], adjtT_ps[:], Sig)
                nc.scalar.activation(adjsT[:], adjsT_ps[:], Sig)

                w_ps = pp.tile([64, 8], _f32, tag=PT[4], name="w_ps")
                nc.tensor.matmul(w_ps[:], xcT[:], sm(_S_WTT, 8))
                u_ps = pp.tile([128, 8], _f32, tag=PT[5], name="u_ps")
                nc.tensor.matmul(u_ps[:], xc_pad[:], sm(_S_WST, 8))
                nc.scalar.activation(w_pad[0:64, :], w_ps[:], Copy)
                nc.scalar.activation(u_sb[:], u_ps[:], Copy)

                tf_ps = pp.tile([64, 8], _f32, tag=PT[6], name="tf_ps")
                nc.tensor.matmul(tf_ps[:], adjtT_pad[:], w_pad[:])
                sf_ps = pp.tile([128, 8], _f32, tag=PT[7], name="sf_ps")
                nc.tensor.matmul(sf_ps[:], adjsT[:], u_sb[:])
                nc.scalar.activation(tf_sb[:], tf_ps[:], Relu)
                nc.scalar.activation(sf_sb[:], sf_ps[:], Relu)

                # vec_tile[p, c] = vec[perm[c*128 + p]]  (host permutes Wout rows to match)
                tf_ev = tf_sb[:].rearrange("p (m two) -> p two m", two=2)
                nc.vector.tensor_copy(out=vec_tile[0:64, 0:4], in_=tf_ev[:, 0, :])
                nc.vector.tensor_copy(out=vec_tile[64:128, 0:4], in_=tf_ev[:, 1, :])
                nc.vector.tensor_copy(out=vec_tile[:, 4:12], in_=sf_sb[:])
                at_ev = adjtT_pad[0:64, :].rearrange("p (m two) -> p two m", two=2)
                nc.vector.tensor_copy(out=vec_tile[0:64, 12:44], in_=at_ev[:, 0, :])
                nc.vector.tensor_copy(out=vec_tile[64:128, 12:44], in_=at_ev[:, 1, :])
                nc.vector.tensor_copy(out=vec_tile[:, 44:172], in_=adjsT[:])

            # ---------- phase 1: out_slice = relu(Wout[rows_k] @ vec) ----------
            outflat = sbf.tile([128, PH2_K], _f32)  # only partition 0 is data
            out_sb = sbf.tile([128, PH2_KC], _f32)
            sig_sb = sbf.tile([128, PH2_KC], _f32)

            with nc.named_scope("ph1"):
                wbig_t = wbig[:].rearrange("(c p) j -> c p j", p=128)
                banks = [
                    pp.tile([1, 512], _f32, tag=PT[n], name=f"b{n}")
                    for n in range(OUT_CHUNKS)
                ]
                widths = [512] * 5 + [RPC - 5 * 512]
                for c in range(C_CHUNKS):
                    strip = sp1.tile([128, RPC], _f32, name="strip")
                    nc.sync.dma_start(out=strip[:], in_=wbig_t[c])
                    for n in range(OUT_CHUNKS):
                        wdt = widths[n]
                        nc.tensor.matmul(
                            banks[n][:, :wdt],
                            vec_tile[:, c:c + 1],
                            strip[:, 512 * n: 512 * n + wdt],
                            start=(c == 0),
                            stop=(c == C_CHUNKS - 1),
                        )
                Relu = mybir.ActivationFunctionType.Relu
                for n in range(OUT_CHUNKS):
                    wdt = widths[n]
                    nc.scalar.activation(
                        outflat[0:1, 512 * n: 512 * n + wdt], banks[n][:, :wdt], Relu
                    )
                nc.vector.memset(outflat[0:1, RPC:PH2_K], 0.0)

            # ---------- transpose out (free-layout -> partition-layout) ----------
            with nc.named_scope("tr"):
                for t in range(PH2_KC):
                    tp = pp.tile([128, 128], _f32, tag=PT[6 if t % 2 == 0 else 7], name="tp")
                    nc.tensor.transpose(
                        tp[:], outflat[:, 128 * t: 128 * (t + 1)], sm(_S_IDENT, 128)
                    )
                    nc.scalar.activation(
                        out_sb[:, t:t + 1], tp[:, 0:1],
                        mybir.ActivationFunctionType.Copy,
                    )
                nc.scalar.activation(
                    sig_sb[:], out_sb[:], mybir.ActivationFunctionType.Sigmoid
                )
                nc.sync.dma_start(out=out_sig[:], in_=sig_sb[:])

            # ---------- phase 2: partial = Wfcn[:, rows_k] @ out_slice ----------
            xout_sb = sbf.tile([1, NF], _f32)
            with nc.named_scope("ph2"):
                wf_t = wf[:].rearrange("(c p) j -> c p j", p=128)
                for half in range(2):
                    b2 = [
                        pp.tile([1, 512], _f32, tag=PT[n], name=f"h{half}b{n}")
                        for n in range(8)
                    ]
                    for c2 in range(PH2_KC):
                        strip2 = sp2.tile([128, 4096], _f32, name="strip2")
                        nc.sync.dma_start(
                            out=strip2[:],
                            in_=wf_t[c2][:, 4096 * half: 4096 * (half + 1)],
                        )
                        for n2 in range(8):
                            nc.tensor.matmul(
                                b2[n2][:, :],
                                out_sb[:, c2:c2 + 1],
                                strip2[:, 512 * n2: 512 * (n2 + 1)],
                                start=(c2 == 0),
                                stop=(c2 == PH2_KC - 1),
                            )
                    for n2 in range(8):
                        nc.vector.tensor_copy(
                            out=xout_sb[0:1, 4096 * half + 512 * n2: 4096 * half + 512 * (n2 + 1)],
                            in_=b2[n2][:, :],
                        )
                nc.sync.dma_start(out=xpart[:], in_=xout_sb[:])

    nc.finalize()
    _nc_cache["nc"] = nc
    return nc


def _perm():
    perm = np.empty(D, dtype=np.int64)
    p = np.arange(128)
    for c in range(4):  # t_f region (64,8) row-major
        perm[c * 128: c * 128 + 64] = 8 * p[:64] + 2 * c
        perm[c * 128 + 64: c * 128 + 128] = 8 * p[:64] + 2 * c + 1
    for c in range(4, 12):  # s_f region (128,8) row-major
        perm[c * 128: (c + 1) * 128] = 512 + 8 * p + (c - 4)
    for c in range(12, 44):  # adj_t region (64,64) row-major
        perm[c * 128: (c + 1) * 128] = 1536 + 128 * (c - 12) + p
    for c in range(44, 172):  # adj_s region (128,128) row-major
        perm[c * 128: (c + 1) * 128] = 5632 + 128 * (c - 44) + p
    return perm


def _pad_rows(a, rows):
    out = np.zeros((rows, a.shape[1]), np.float32)
    out[: a.shape[0]] = a
    return out


def kernel(x, t_adj_hyp, s_adj_hyp, Wc, Wt, Wtadj, Ws, Wsadj, Wout, Wfcn, bfcn, curvs):
    x = np.nan_to_num(np.asarray(x, np.float32))
    t_adj_hyp = np.asarray(t_adj_hyp, np.float32)
    s_adj_hyp = np.asarray(s_adj_hyp, np.float32)
    Wc = np.asarray(Wc, np.float32)
    Wt = np.asarray(Wt, np.float32)
    Wtadj = np.asarray(Wtadj, np.float32)
    Ws = np.asarray(Ws, np.float32)
    Wsadj = np.asarray(Wsadj, np.float32)
    Wout = np.asarray(Wout, np.float32)
    Wfcn = np.asarray(Wfcn, np.float32)
    bfcn = np.asarray(bfcn, np.float32)
    curvs_np = np.asarray(curvs, np.float32)

    smalls = np.zeros((128, _S_COLS), np.float32)
    smalls[:, _S_XT:_S_XT + 64] = x.T
    smalls[:, _S_WCT:_S_WCT + 128] = Wc.T
    smalls[:, _S_WTT:_S_WTT + 8] = Wt.T
    smalls[:, _S_WST:_S_WST + 8] = _pad_rows(Ws.T, 128)
    smalls[:, _S_WTADJT:_S_WTADJT + 64] = _pad_rows(Wtadj.T, 128)
    smalls[:, _S_TADJT:_S_TADJT + 64] = _pad_rows(t_adj_hyp.T, 128)
    smalls[:, _S_WSADJT:_S_WSADJT + 128] = Wsadj.T
    smalls[:, _S_SADJT:_S_SADJT + 128] = s_adj_hyp.T
    smalls[:, _S_IDENT:_S_IDENT + 128] = np.eye(128, dtype=np.float32)

    perm = _perm()
    in_maps = []
    for k in range(NCORES):
        j_lo, j_hi = k * RPC, (k + 1) * RPC
        wbig_k = np.ascontiguousarray(Wout[j_lo:j_hi].take(perm, axis=1).T)
        wf_k = np.zeros((PH2_K, NF), np.float32)
        wf_k[:RPC] = Wfcn[:, j_lo:j_hi].T
        in_maps.append({"smalls": smalls, "wbig": wbig_k, "wf": wf_k})

    global _last_in_maps
    _last_in_maps = in_maps

    nc = _build_nc()
    res = bass_utils.run_bass_kernel_spmd(
        nc, in_maps, core_ids=list(range(NCORES)), trace=False
    )

    sig_full = np.concatenate(
        [res.results[k]["out_sig"].T.reshape(PH2_K)[:RPC] for k in range(NCORES)]
    )
    t_out = sig_full[CD * N + CD * F: CD * N + CD * F + N * N].reshape(N, N)
    s_out = sig_full[-F * F:].reshape(F, F)

    xsum = np.zeros(NF, np.float32)
    for k in range(NCORES):
        xsum += res.results[k]["xpart"][0]
    x_out = np.maximum(xsum + bfcn, 0.0).astype(np.float32)

    return (x_out, t_out, s_out, curvs_np)
